# revision 53
# baseline (speedup 1.0000x reference)
"""Trainium2 Bass kernel for a hybrid attention+SwiGLU transformer layer.

Strategy: pure data parallelism over B*S = 4096 tokens -> 8 shards of 512.
Each core recomputes K/V over a 1024-token halo (sliding-window attention),
so no collectives are needed. Activations are kept feature-major ("transposed",
[feature, token]) on chip so every projection matmul uses the natural weight
layout as the stationary operand and tokens as the moving free dim (N=512).
Attention scores are computed transposed (scoresT[k, q]) which makes the
whole attention block transpose-free; softmax sums over the partition axis
via ones-matmuls on the PE.

v2 restructure (vs baseline):
 - hT (normed/transposed activations) lives entirely in SBUF; the DRAM
   round-trip and its thousands of tiny descriptors are gone.
 - Phase order P1-own -> Q -> P1-ctx -> KV so Q's matmuls overlap the
   context-halo rmsnorm work.
 - Attention processes 4 heads of one kv-group together, software-pipelined
   by one k-tile so the PE never waits on exp/mask; softmax sums for the 4
   heads pack into one PSUM bank (partition rows 0/32/64/96); interior
   k-tiles skip the mask multiply entirely (per-tile exp bias handles the
   all-or-nothing cases).
 - O-projection loops dc-outer so wo streams exactly once (8MB not 32MB).
 - x2 (post-attention residual) lives in SBUF; n2/fin read it directly.

Matmuls run in bf16 with fp32 PSUM accumulation; norms/softmax/residuals in
fp32. Weights are pre-cast/pre-tiled on host; rotary tables, window masks and
per-tile exp biases are host-precomputed per core.

v3: Q/K/V/O projections, attention AV and the softmax-sum matmuls run in
fp8e4 with DoubleRow perf mode (two contraction rows per PE pass): hT, vP,
attnT and the exp outputs are stored fp8; wq/wk are host-scaled by 64 into
fp8's normal range and descaled for free via the existing rope-path copies.
Scores (q.k) and the whole FFN stay bf16 for accuracy (fp8 FFN busts the
2e-2 tolerance; measured in sim).

v4-v10 pipeline work:
 - sliding-window-aware edges: k-tiles 0/1 (10/11) only score/exp the low
   (high) query half; the dead pm half is memset once.
 - x loads once into the x2 residual buffer (P1-own norms read it, the O
   projection accumulates into it in place); P1-ctx DMA is prefetch-queued.
 - attention groups software-pipeline ACROSS groups: the last two AV/ssum
   pair-drains and the accumulator copy-out are carried into the next
   group's loop (issued behind its independent score matmuls), PSUM state
   is allocated lazily, and softmax normalization is PE-free: one batched
   DVE reciprocal, 1/sum row partition-broadcast by the idle DMA engines
   via a DRAM bounce, final scale on DVE.  Full-width boundary tiles of
   head j=0 use a Schraudolph fast-exp on DVE (fused with the mask mul)
   to offload the scalar engine's exp wall.
 - n2 rmsnorm tiles are emitted as their x2 rows finish; the FFN output
   transposes/stores interleave with the last down-proj matmuls.
"""
import sys, os, math

sys.path.insert(0, '/opt/trn_rl_repo')

import numpy as np
import ml_dtypes

import concourse.bass as bass
import concourse.mybir as mybir
import concourse.tile as tile
from concourse import bacc
from concourse.masks import make_identity
from concourse.bass_utils import run_bass_kernel_spmd

AF = mybir.ActivationFunctionType
DT = mybir.dt
ALU = mybir.AluOpType
BF16 = ml_dtypes.bfloat16

N_CORES = 8
EPS = 1e-6
ROPE_BASE = 10000.0
RD = 64           # rotary dim
WINDOW = 1024
EXP_BIAS = -3.5
QK_SCALE = 64.0   # host-side wq/wk scale into fp8 normal range
F8 = ml_dtypes.float8_e4m3

FULL = dict(D=2048, H=16, KVH=4, FFN=8192, B=2, S=2048, OWN=512, CTX=1536)

# boundary context tiles (those that ever need an elementwise mask)
BOUND = [0, 1, 2, 3, 8, 9, 10, 11]

# quadrant-local 16-row half swap for stream_shuffle (rope pair exchange)
SHUF_MASK = [(i + 16) % 32 for i in range(32)]
# per-head rotary feature permutation: [e0..e15 | o0..o15 | e16..e31 | o16..o31 | 64:]
ROPE_PERM = ([2 * i for i in range(16)] + [2 * i + 1 for i in range(16)]
             + [32 + 2 * i for i in range(16)] + [33 + 2 * i for i in range(16)]
             + list(range(64, 128)))


def build_program(cfg):
    D, H, KVH, FFN = cfg['D'], cfg['H'], cfg['KVH'], cfg['FFN']
    OWN, CTX = cfg['OWN'], cfg['CTX']
    HD = 128
    ND = D // 128            # feature blocks of the model dim
    NF = FFN // 128          # feature blocks of the ffn dim
    NO = OWN // 128          # own token tiles (4)
    NT = CTX // 128          # context token tiles (12)
    NCH = CTX // 512         # context chunks of 512 (3)
    NB = len(BOUND)          # boundary tiles (8)
    FG = 16
    NFG = NF // FG
    VW = KVH * HD
    REP = H // KVH
    f32, bf16, f8 = DT.float32, DT.bfloat16, DT.float8e4
    DR = mybir.MatmulPerfMode.DoubleRow
    rsd = 1.0 / math.sqrt(HD)

    nc = bacc.Bacc("TRN2", target_bir_lowering=False, debug=False,
                   num_devices=N_CORES)

    # ---------------- DRAM I/O ----------------
    x_ctx = nc.dram_tensor("x_ctx", [CTX, D], f32, kind="ExternalInput")
    wq_d = nc.dram_tensor("wq", [H, 128, ND, 128], f8, kind="ExternalInput")
    wk_d = nc.dram_tensor("wk", [KVH, 128, ND, 128], f8, kind="ExternalInput")
    wv_d = nc.dram_tensor("wv", [128, ND, VW], f8, kind="ExternalInput")
    wo_d = nc.dram_tensor("wo", [H // 2, 128, 2, D], f8, kind="ExternalInput")
    wg_d = nc.dram_tensor("wg", [NF, 128, ND * 128], bf16, kind="ExternalInput")
    wu_d = nc.dram_tensor("wu", [NF, 128, ND * 128], bf16, kind="ExternalInput")
    wd_d = nc.dram_tensor("wd", [ND, 128, NF * 128], bf16, kind="ExternalInput")
    cosq_d = nc.dram_tensor("cosq", [64, OWN], bf16, kind="ExternalInput")
    sinq_d = nc.dram_tensor("sinq", [64, OWN], bf16, kind="ExternalInput")
    cosk_d = nc.dram_tensor("cosk", [64, CTX], bf16, kind="ExternalInput")
    sink_d = nc.dram_tensor("sink", [64, CTX], bf16, kind="ExternalInput")
    mask_d = nc.dram_tensor("mask", [128, NB * OWN], f8, kind="ExternalInput")
    bias_d = nc.dram_tensor("biast", [128, NT], f32, kind="ExternalInput")
    biasf_d = nc.dram_tensor("biasf", [128, NT], f32, kind="ExternalInput")
    y_d = nc.dram_tensor("y", [OWN, D], f32, kind="ExternalOutput")
    EXPS = 12102203.161561486        # 2^23 / ln 2 (Schraudolph fast exp)

    from contextlib import ExitStack
    with tile.TileContext(nc) as tc:
        with ExitStack() as ctx:
            pool = lambda *a, **kw: ctx.enter_context(tc.tile_pool(*a, **kw))
            constp = pool(name="const", bufs=1)
            costp = pool(name="cost", bufs=1)     # cos/sin tables
            maskp = pool(name="maskp", bufs=1)
            qTp = pool(name="qT", bufs=1)
            kTp = pool(name="kT", bufs=1)
            vPp = pool(name="vP", bufs=1)
            wpanp = pool(name="wpan", bufs=4)     # streamed weight panels
            smlp = pool(name="sml", bufs=2)
            psp = pool(name="ps", bufs=8, space="PSUM")

            identity_bf = constp.tile([128, 128], bf16, tag="idb")
            make_identity(nc, identity_bf[:])
            identity_f32 = constp.tile([128, 128], f32, tag="idf")
            make_identity(nc, identity_f32[:])
            ones_col = constp.tile([128, 2, 16], f8, tag="ones_col")
            nc.gpsimd.memset(ones_col[:], 1.0)
            ones_row = constp.tile([1, 128], bf16, tag="ones_row")
            nc.gpsimd.memset(ones_row[:], 1.0)
            eps_b = constp.tile([128, 1], f32, tag="eps_b")
            nc.gpsimd.memset(eps_b[:], EPS)

            # rope tables, masks, biases
            cosq = costp.tile([64, OWN], bf16, tag="cq")
            sinq = costp.tile([64, OWN], bf16, tag="sq")
            cosk = costp.tile([64, CTX], bf16, tag="ck")
            sink = costp.tile([64, CTX], bf16, tag="sk")
            nc.sync.dma_start(cosq[:], cosq_d[:])
            nc.sync.dma_start(sinq[:], sinq_d[:])
            nc.sync.dma_start(cosk[:], cosk_d[:])
            nc.sync.dma_start(sink[:], sink_d[:])
            bslot = {t: bi for bi, t in enumerate(BOUND)}

            qT = qTp.tile([128, H * OWN], bf16, tag="qT")
            kT = kTp.tile([128, KVH * CTX], bf16, tag="kT")
            vP = vPp.tile([128, NT, VW], f8, tag="vP")

            # x2 is the post-attention residual buffer; it starts life as the
            # own x rows (loaded once up front, used by P1's own-tile norms),
            # then the O projection accumulates into it in place.
            x2p = pool(name="x2", bufs=1)
            x2 = x2p.tile([128, NO * D], f32, tag="x2")
            for mt in range(NO):
                nc.sync.dma_start(x2[:, mt * D:(mt + 1) * D],
                                  x_ctx[(NT - NO + mt) * 128:
                                        (NT - NO + mt + 1) * 128, :])

            # ====================================================
            # Scope 1: P1-own -> Q -> P1-ctx -> KV   (hT resident)
            # ====================================================
            with tc.tile_pool(name="hT", bufs=1) as hTp, \
                 tc.tile_pool(name="kpan", bufs=4) as kpanp, \
                 tc.tile_pool(name="wvp", bufs=3) as wvp, \
                 tc.tile_pool(name="xf32", bufs=2) as xf32p, \
                 tc.tile_pool(name="hbf", bufs=4) as hbfp, \
                 tc.tile_pool(name="rope", bufs=2) as ropep:

                hT = hTp.tile([128, ND, CTX], f8, tag="hT")

                p1_pre = {}

                def p1_dma(i):
                    """prefetch ctx tile i from DRAM (own tiles live in x2)"""
                    if i >= NT - NO or i in p1_pre:
                        return
                    xt = xf32p.tile([128, D], f32, tag="xf32", bufs=3)
                    nc.sync.dma_start(xt[:], x_ctx[i * 128:(i + 1) * 128, :])
                    p1_pre[i] = xt

                def p1_tile(i):
                    """rmsnorm + transpose ctx tile i into hT.

                    Transposed blocks land packed 8-per-PSUM-bank; the two
                    bank-wide copies out split across scalar and vector so
                    neither engine becomes the per-tile bottleneck.
                    """
                    if i >= NT - NO:
                        xta = x2[:, (i - (NT - NO)) * D:(i - (NT - NO) + 1) * D]
                    else:
                        p1_dma(i)
                        xta = p1_pre.pop(i)[:]
                        p1_dma(i + 1)
                        p1_dma(i + 2)
                    sq = hbfp.tile([128, D], bf16, tag="hbf")
                    ss = smlp.tile([128, 1], f32, tag="ss")
                    nc.scalar.activation(sq[:], xta, AF.Square, accum_out=ss[:])
                    sr = smlp.tile([128, 1], f32, tag="sr")
                    nc.scalar.activation(sr[:], ss[:], AF.Sqrt, scale=1.0 / D,
                                         bias=eps_b[:])
                    rr = smlp.tile([128, 1], f32, tag="rr")
                    nc.vector.reciprocal(rr[:], sr[:])
                    ht = hbfp.tile([128, D], bf16, tag="hbf")
                    nc.vector.tensor_scalar_mul(ht[:], xta, rr[:])
                    # hT free layout within a db block is i*128; 8 consecutive
                    # db blocks of one token tile are NOT contiguous in hT, so
                    # copy out per-db slices from the packed bank.
                    for half in range(2):
                        ptr = psp.tile([128, 1024], bf16, tag="ps")
                        for k in range(8):
                            db = half * 8 + k
                            nc.tensor.transpose(
                                ptr[:, k * 128:(k + 1) * 128],
                                ht[:, db * 128:(db + 1) * 128],
                                identity_bf[:])
                        for k in range(8):
                            db = half * 8 + k
                            dst = hT[:, db, i * 128:(i + 1) * 128]
                            if half == 0:
                                nc.scalar.copy(dst, ptr[:, k * 128:(k + 1) * 128])
                            else:
                                nc.vector.tensor_copy(dst, ptr[:, k * 128:(k + 1) * 128])

                # ---- P1 own tiles (ctx tiles 8..11) ----
                for mt in range(NO):
                    p1_tile(NT - NO + mt)
                for i in range(2):       # prime the ctx-tile prefetch queue
                    p1_dma(i)

                # ---- Q projection + rope, interleaved with P1 ctx tiles so
                # ---- the halo rmsnorm hides under Q's matmul chains ----
                OFF = CTX - OWN
                DSC = 1.0 / QK_SCALE
                for hb in range(H):
                    pan = wpanp.tile([128, ND, 128], f8, tag="wpan")
                    nc.sync.dma_start(pan[:], wq_d[hb])
                    pq = psp.tile([128, OWN], f32, tag="ps")
                    for db in range(0, ND, 2):
                        nc.tensor.matmul(
                            pq[:], pan[:, db:db + 2, :],
                            hT[:, db:db + 2, OFF:OFF + OWN],
                            start=(db == 0), stop=(db == ND - 2),
                            perf_mode=DR)
                    qsl = qT[:, hb * OWN:(hb + 1) * OWN]
                    qstage = ropep.tile([64, OWN], bf16, tag="rst")
                    nc.scalar.mul(qstage[:], pq[0:64, :], DSC)
                    shuf = ropep.tile([64, OWN], bf16, tag="rsh")
                    nc.vector.stream_shuffle(shuf[:], qstage[:], SHUF_MASK)
                    t1 = ropep.tile([64, OWN], bf16, tag="rt1", bufs=1)
                    nc.vector.tensor_mul(t1[:], qstage[:], cosq[:])
                    t2 = ropep.tile([64, OWN], bf16, tag="rt2", bufs=1)
                    nc.vector.tensor_mul(t2[:], shuf[:], sinq[:])
                    nc.vector.tensor_add(qsl[0:64, :], t1[:], t2[:])
                    nc.scalar.mul(qsl[64:128, :], pq[64:128, :], DSC)
                    if hb < NT - NO:
                        p1_tile(hb)

                # ---- K + V over the full context (from hT in SBUF) ----
                kpan = []
                for kb in range(KVH):
                    kp = kpanp.tile([128, ND, 128], f8, tag="kpan")
                    nc.sync.dma_start(kp[:], wk_d[kb])
                    kpan.append(kp)
                for ch in range(NCH):
                    pk = [psp.tile([128, 512], f32, tag="ps", name=f"pk{ch}_{kb}")
                          for kb in range(KVH)]
                    pv = [psp.tile([128, VW], f32, tag="ps", name=f"pv{ch}_{mi}")
                          for mi in range(4)]
                    for db in range(0, ND, 2):
                        hsl = hT[:, db:db + 2, ch * 512:(ch + 1) * 512]
                        wvs = wvp.tile([128, 2, VW], f8, tag="wvs")
                        nc.sync.dma_start(wvs[:], wv_d[:, db:db + 2, :])
                        for kb in range(KVH):
                            nc.tensor.matmul(pk[kb][:],
                                             kpan[kb][:, db:db + 2, :],
                                             hsl,
                                             start=(db == 0), stop=(db == ND - 2),
                                             perf_mode=DR)
                        for mi in range(4):
                            nc.tensor.matmul(pv[mi][:],
                                             hT[:, db:db + 2,
                                                ch * 512 + mi * 128:
                                                ch * 512 + (mi + 1) * 128],
                                             wvs[:],
                                             start=(db == 0), stop=(db == ND - 2),
                                             perf_mode=DR)
                    for mi in range(4):
                        t_idx = ch * 4 + mi
                        nc.vector.tensor_copy(vP[:, t_idx, :], pv[mi][:])
                    for kb in range(KVH):
                        ksl = kT[:, kb * CTX + ch * 512: kb * CTX + (ch + 1) * 512]
                        kstage = ropep.tile([64, 512], bf16, tag="rst")
                        nc.scalar.mul(kstage[:], pk[kb][0:64, :], DSC)
                        shuf = ropep.tile([64, 512], bf16, tag="rsh")
                        nc.vector.stream_shuffle(shuf[:], kstage[:], SHUF_MASK)
                        t1 = ropep.tile([64, 512], bf16, tag="rt1", bufs=1)
                        nc.vector.tensor_mul(t1[:], kstage[:],
                                             cosk[:, ch * 512:(ch + 1) * 512])
                        t2 = ropep.tile([64, 512], bf16, tag="rt2", bufs=1)
                        nc.vector.tensor_mul(t2[:], shuf[:],
                                             sink[:, ch * 512:(ch + 1) * 512])
                        nc.vector.tensor_add(ksl[0:64, :], t1[:], t2[:])
                        nc.scalar.mul(ksl[64:128, :], pk[kb][64:128, :], DSC)

            # ====================================================
            # Attention: 4 heads per kv-group, pipelined by one k-tile
            # ====================================================
            bigB = pool(name="bigB", bufs=1)      # attnT -> gT
            ppp = pool(name="pp", bufs=8)         # small bf16 [128,OWN] tiles
            pbp = pool(name="pbp", bufs=2)        # broadcast 1/ssum tiles
            drp = pool(name="drp", bufs=4, space="DRAM")
            osbp = pool(name="osb", bufs=2)
            stgp = pool(name="stg", bufs=3)       # [128,512] staging
            recpp = pool(name="recp", bufs=2)
            masks = maskp.tile([128, NB * OWN], f8, tag="mask")
            nc.sync.dma_start(masks[:], mask_d[:])
            biast = maskp.tile([128, NT], f32, tag="biast")
            nc.sync.dma_start(biast[:], bias_d[:])
            biasf = maskp.tile([128, NT], f32, tag="biasf")
            nc.sync.dma_start(biasf[:], biasf_d[:])

            attnT = bigB.tile([128, H, OWN], f8, tag="bigB")
            GW = 2                    # heads processed together
            NG = H // GW
            NTP = NT // 2             # context tile pairs (DoubleRow AV)
            # sliding window: query half m01 only sees k-tiles 0..9, half
            # m23 only 2..11 -> edge tiles compute half-width scores/exp and
            # the dead pm half is zeroed once so full-width AV/ssum see 0.
            QR = {0: (0, 256), 1: (0, 256),
                  10: (256, 512), 11: (256, 512)}
            carry = []                # prev group's tail drains (closures)
            deferred = [None, None]   # prev group's finalize closures (a, b)
            for grp in range(NG):
                kb = (grp * GW) // REP
                heads = [grp * GW + j for j in range(GW)]
                # PSUM state allocated lazily (first own drain) so the
                # previous group's banks can free first
                st = {}
                pending = []        # [(tp, [pm pair tiles])] awaiting AV/ssum
                def drain_one(last=False, _st=st, _kb=kb, _pending=pending,
                              _g=grp):
                    if 'ap' not in _st:
                        _st['ap'] = [psp.tile([128, OWN], f32, tag="ps",
                                              name=f"ap{_g % 2}_{j}")
                                     for j in range(GW)]
                        _st['ss'] = [psp.tile([1, OWN], f32, tag="ps",
                                              name=f"ssum{_g % 2}_{j}")
                                     for j in range(GW)]
                    tp_, pms = _pending.pop(0)
                    for j in range(GW):
                        vsl = vP[:, 2 * tp_:2 * tp_ + 2, _kb * HD:(_kb + 1) * HD]
                        nc.tensor.matmul(
                            _st['ap'][j][:], vsl, pms[j][:], start=(tp_ == 0),
                            stop=(last and tp_ == NTP - 1), perf_mode=DR)
                        nc.tensor.matmul(
                            _st['ss'][j][:], ones_col[:, :, 0:1],
                            pms[j][:], start=(tp_ == 0),
                            stop=(last and tp_ == NTP - 1), perf_mode=DR)
                for tp in range(NTP):
                    pms = [ppp.tile([128, 2, OWN], f8, tag="pt", name=f"pm{j}")
                           for j in range(GW)]
                    # zero the window-dead half of the edge pairs
                    for j in range(GW):
                        if tp == 0:
                            nc.vector.memset(pms[j][:, :, 256:512], 0.0)
                        elif tp == NTP - 1:
                            nc.vector.memset(pms[j][:, :, 0:256], 0.0)
                    for i in range(2):
                        t = 2 * tp + i
                        q0, q1 = QR.get(t, (0, 512))
                        qw = q1 - q0
                        sps = []
                        for j, hb in enumerate(heads):
                            sp = psp.tile([128, OWN], f32, tag="ps")
                            nc.tensor.matmul(
                                sp[:, 0:qw],
                                kT[:, kb * CTX + t * 128: kb * CTX + (t + 1) * 128],
                                qT[:, hb * OWN + q0: hb * OWN + q1],
                                start=True, stop=True)
                            sps.append(sp)
                        for j in range(GW):
                            if t in bslot:
                                if j == 0 and qw == 512:
                                    # Schraudolph fast exp on DVE, fused with
                                    # the mask multiply: int(x*S+B) bitcast
                                    # f32 ~= e^x within ~2%
                                    ti = ppp.tile([128, OWN], DT.int32,
                                                  tag="ti", bufs=2)
                                    nc.vector.tensor_scalar(
                                        ti[:, 0:qw], sps[j][:, 0:qw],
                                        rsd * EXPS, biasf[:, t:t + 1],
                                        ALU.mult, ALU.add)
                                    nc.vector.tensor_mul(
                                        pms[j][:, i, q0:q1],
                                        ti[:, 0:qw].bitcast(f32),
                                        masks[:, bslot[t] * OWN + q0:
                                              bslot[t] * OWN + q1])
                                else:
                                    pt = ppp.tile([128, OWN], bf16, tag="ptb",
                                                  bufs=3)
                                    nc.scalar.activation(
                                        pt[:, 0:qw], sps[j][:, 0:qw],
                                        AF.Exp, scale=rsd,
                                        bias=biast[:, t:t + 1])
                                    nc.vector.tensor_mul(
                                        pms[j][:, i, q0:q1], pt[:, 0:qw],
                                        masks[:, bslot[t] * OWN + q0:
                                              bslot[t] * OWN + q1])
                            else:
                                nc.scalar.activation(pms[j][:, i, :], sps[j][:],
                                                     AF.Exp, scale=rsd,
                                                     bias=biast[:, t:t + 1])
                    # interleave after this pair's (independent) scores:
                    # prev group's tail drains, its finalize, own lag-2 drain
                    if carry:
                        carry.pop(0)()
                    else:
                        if tp == 2 and deferred[0] is not None:
                            deferred[0]()
                            deferred[0] = None
                        if tp == 4 and deferred[1] is not None:
                            deferred[1]()
                            deferred[1] = None
                        if len(pending) == 2:
                            drain_one()
                    pending.append((tp, pms))
                # drain down to two pairs; their exps are still in flight, so
                # defer them (and the accumulator/sum copy-out) into the next
                # group's loop where its scores keep the PE busy meanwhile
                while len(pending) > 2:
                    drain_one()
                asbs = [osbp.tile([128, OWN], bf16, tag="osb",
                                  name=f"asb{grp % 2}_{j}")
                        for j in range(GW)]
                ssc = recpp.tile([33, OWN], f32, tag="ssc",
                                 name=f"ssc{grp % 2}")

                def tail1(d=drain_one):
                    d()
                def tail2(d=drain_one, _st=st, _asbs=asbs, _ssc=ssc):
                    d(last=True)
                    # copy PSUM state out right away: frees all 4 banks for
                    # the next group; both sums land quadrant-aligned in one
                    # tile so one per-partition-serial reciprocal serves both
                    for j in range(GW):
                        nc.vector.tensor_copy(_asbs[j][:], _st['ap'][j][:])
                        nc.scalar.copy(_ssc[32 * j:32 * j + 1, :],
                                       _st['ss'][j][:])
                carry = [tail1, tail2]

                # normalization, PE-free: reciprocal on DVE, then the
                # per-query 1/sum row is partition-broadcast by the (idle)
                # DMA engines via a DRAM bounce; final mul on DVE
                pbbs = [pbp.tile([128, OWN], bf16, tag="pbb",
                                 name=f"pbb{grp % 2}_{j}")
                        for j in range(GW)]

                def make_fina(ssc=ssc, pbbs=pbbs, grp=grp):
                    def fina():
                        rec = recpp.tile([33, OWN], bf16, tag="rec", bufs=2)
                        with nc.allow_low_precision(
                                reason="1/ssum broadcast in bf16 is plenty"):
                            nc.vector.reciprocal(rec[:], ssc[:])
                        for j in range(GW):
                            recd = drp.tile([1, OWN], bf16, tag="recd",
                                            name=f"recd{grp % 2}_{j}")
                            nc.sync.dma_start(recd[:],
                                              rec[32 * j:32 * j + 1, :])
                            nc.sync.dma_start(
                                pbbs[j][:], recd[:].to_broadcast((128, OWN)))
                    return fina

                def make_finb(heads=heads, asbs=asbs, pbbs=pbbs):
                    def finb():
                        for j, hb in enumerate(heads):
                            nc.vector.tensor_mul(
                                attnT[:, hb, :], asbs[j][:], pbbs[j][:])
                    return finb
                deferred = [make_fina(), make_finb()]
            for c in carry:
                c()
            deferred[0]()
            deferred[1]()
            deferred = [None, None]

            # ====================================================
            # Scope 2: O projection (+residual) -> x2 (SBUF), n2, FFN, fin
            # ====================================================
            with tc.tile_pool(name="hbf2", bufs=2) as hbfp2:
                gT = bigB.tile([128, ND * OWN], bf16, tag="bigB")

                def n2_tile(mt):
                    """rmsnorm + transpose x2 row-block mt -> gT."""
                    x2t = x2[:, mt * D:(mt + 1) * D]
                    sq = hbfp2.tile([128, D], bf16, tag="hbf")
                    ss = smlp.tile([128, 1], f32, tag="ss")
                    nc.scalar.activation(sq[:], x2t, AF.Square, accum_out=ss[:])
                    sr = smlp.tile([128, 1], f32, tag="sr")
                    nc.scalar.activation(sr[:], ss[:], AF.Sqrt, scale=1.0 / D,
                                         bias=eps_b[:])
                    rr = smlp.tile([128, 1], f32, tag="rr")
                    nc.vector.reciprocal(rr[:], sr[:])
                    gt = hbfp2.tile([128, D], bf16, tag="hbf")
                    nc.vector.tensor_scalar_mul(gt[:], x2t, rr[:])
                    for half in range(2):
                        ptr = psp.tile([128, 1024], bf16, tag="ps")
                        for k in range(8):
                            db = half * 8 + k
                            nc.tensor.transpose(
                                ptr[:, k * 128:(k + 1) * 128],
                                gt[:, db * 128:(db + 1) * 128],
                                identity_bf[:])
                        for k in range(8):
                            db = half * 8 + k
                            dst = gT[:, db * OWN + mt * 128:
                                     db * OWN + (mt + 1) * 128]
                            if half == 0:
                                nc.scalar.copy(dst, ptr[:, k * 128:(k + 1) * 128])
                            else:
                                nc.vector.tensor_copy(dst, ptr[:, k * 128:(k + 1) * 128])

                NDC = D // 512
                for dc in range(NDC):
                    pos = [psp.tile([128, 512], f32, tag="ps",
                                    name=f"po{dc % 2}_{mt}")
                           for mt in range(NO)]
                    for hp in range(H // 2):
                        pan = wpanp.tile([128, 2, 512], f8, tag="wopan")
                        nc.sync.dma_start(
                            pan[:], wo_d[hp][:, :, dc * 512:(dc + 1) * 512])
                        for mt in range(NO):
                            nc.tensor.matmul(
                                pos[mt][:],
                                attnT[:, 2 * hp:2 * hp + 2,
                                      mt * 128:(mt + 1) * 128],
                                pan[:],
                                start=(hp == 0), stop=(hp == H // 2 - 1),
                                perf_mode=DR)
                    for mt in range(NO):
                        xsl = x2[:, mt * D + dc * 512: mt * D + (dc + 1) * 512]
                        nc.vector.tensor_add(xsl, pos[mt][:], xsl)
                        # rmsnorm+transpose of a finished token block overlaps
                        # the remaining adds / FFN weight prefetch
                        if dc == NDC - 1:
                            n2_tile(mt)

                # ---- FFN gate/up/down ----
                with tc.tile_pool(name="acc", bufs=1) as accp, \
                     tc.tile_pool(name="tfg", bufs=1) as tfgp:
                    acc = accp.tile([128, ND * OWN], f32, tag="acc")

                    def fin_og(og):
                        """transpose + final residual -> y for 4 acc blocks;
                        interleaved with the last fg group's down matmuls."""
                        for mt in range(NO):
                            ptg = psp.tile([128, 512], f32, tag="ps")
                            for k in range(4):
                                ob = og * 4 + k
                                nc.tensor.transpose(
                                    ptg[:, k * 128:(k + 1) * 128],
                                    acc[:, ob * OWN + mt * 128:
                                        ob * OWN + (mt + 1) * 128],
                                    identity_f32[:])
                            ys = stgp.tile([128, 512], f32, tag="ys", bufs=2)
                            nc.vector.tensor_add(
                                ys[:], ptg[:],
                                x2[:, mt * D + og * 512: mt * D + (og + 1) * 512])
                            nc.sync.dma_start(
                                y_d[mt * 128:(mt + 1) * 128,
                                    og * 512:(og + 1) * 512], ys[:])

                    for fg in range(NFG):
                        t_fg = tfgp.tile([128, FG * OWN], bf16, tag="tfg")
                        for j in range(FG):
                            fb = fg * FG + j
                            gpan = wpanp.tile([128, ND * 128], bf16, tag="wpan")
                            nc.sync.dma_start(gpan[:], wg_d[fb])
                            upan = wpanp.tile([128, ND * 128], bf16, tag="wpan")
                            nc.sync.dma_start(upan[:], wu_d[fb])
                            pg = psp.tile([128, OWN], f32, tag="ps")
                            pu = psp.tile([128, OWN], f32, tag="ps")
                            for db in range(ND):
                                nc.tensor.matmul(pg[:], gpan[:, db * 128:(db + 1) * 128],
                                                 gT[:, db * OWN:(db + 1) * OWN],
                                                 start=(db == 0), stop=(db == ND - 1))
                                nc.tensor.matmul(pu[:], upan[:, db * 128:(db + 1) * 128],
                                                 gT[:, db * OWN:(db + 1) * OWN],
                                                 start=(db == 0), stop=(db == ND - 1))
                            sg = osbp.tile([128, OWN], bf16, tag="osb")
                            nc.scalar.activation(sg[:], pg[:], AF.Sigmoid)
                            sg2 = ppp.tile([128, OWN], bf16, tag="pt")
                            nc.vector.tensor_mul(sg2[:], sg[:], pg[:])
                            nc.vector.tensor_mul(t_fg[:, j * OWN:(j + 1) * OWN],
                                                 sg2[:], pu[:])
                        for ob in range(ND):
                            dpan = wpanp.tile([128, FG * 128], bf16, tag="wpan")
                            nc.sync.dma_start(
                                dpan[:], wd_d[ob, :, fg * FG * 128:(fg + 1) * FG * 128])
                            pd = psp.tile([128, OWN], f32, tag="ps")
                            for j in range(FG):
                                nc.tensor.matmul(pd[:], dpan[:, j * 128:(j + 1) * 128],
                                                 t_fg[:, j * OWN:(j + 1) * OWN],
                                                 start=(j == 0), stop=(j == FG - 1))
                            osl = acc[:, ob * OWN:(ob + 1) * OWN]
                            if fg == 0:
                                nc.scalar.copy(osl, pd[:])
                            else:
                                nc.vector.tensor_add(osl, osl, pd[:])
                                if fg == NFG - 1 and ob % 4 == 3:
                                    fin_og(ob // 4)

    nc.compile()
    return nc


# ---------------------------------------------------------------------------
# Host-side preparation
# ---------------------------------------------------------------------------

def _rope_tables(pos, dtype=BF16):
    """Build the [64, m] A (cos) and B (+-sin) tables for the permuted layout."""
    inv_freq = 1.0 / (ROPE_BASE ** (np.arange(0, RD, 2, dtype=np.float64) / RD))
    ang = inv_freq[:, None] * pos[None, :].astype(np.float64)   # [32, m]
    cos, sin = np.cos(ang), np.sin(ang)
    rmap = np.concatenate([np.arange(16), np.arange(16),
                           np.arange(16, 32), np.arange(16, 32)])
    sign = np.ones(64); sign[0:16] = -1.0; sign[32:48] = -1.0
    A = cos[rmap]                       # [64, m]
    B = sign[:, None] * sin[rmap]
    return A.astype(dtype), B.astype(dtype)


def prep_inputs(cfg, x, position_ids, attn_norm_w, wq, wk, wv, wo, ffn_norm_w,
                w_gate, w_up, w_down):
    D, H, KVH, FFN = cfg['D'], cfg['H'], cfg['KVH'], cfg['FFN']
    B, S, OWN, CTX = cfg['B'], cfg['S'], cfg['OWN'], cfg['CTX']
    HD = 128
    ND, NF, NT = D // 128, FFN // 128, CTX // 128
    NCHUNK = S // OWN

    x = np.asarray(x, np.float32)
    anw = np.asarray(attn_norm_w, np.float32)
    fnw = np.asarray(ffn_norm_w, np.float32)
    perm = np.asarray(ROPE_PERM)

    def panelize(w, nout):
        # w: [D_in, NOUT*128] -> [NOUT, 128, ND_in*128] panel image
        din = w.shape[0]
        ndin = din // 128
        return np.ascontiguousarray(
            w.reshape(ndin, 128, nout, 128).transpose(2, 1, 0, 3)
            .reshape(nout, 128, ndin * 128))

    wq_f = (np.asarray(wq, np.float32) * anw[:, None]).reshape(D, H, HD)
    wq_f = wq_f[:, :, perm].reshape(D, H * HD) * QK_SCALE
    wq_t = panelize(wq_f, H).astype(F8).reshape(H, 128, ND, 128)
    wk_f = (np.asarray(wk, np.float32) * anw[:, None]).reshape(D, KVH, HD)
    wk_f = wk_f[:, :, perm].reshape(D, KVH * HD) * QK_SCALE
    wk_t = panelize(wk_f, KVH).astype(F8).reshape(KVH, 128, ND, 128)
    VW = KVH * HD
    wv_f = np.asarray(wv, np.float32) * anw[:, None]
    wv_t = np.ascontiguousarray(
        wv_f.reshape(ND, 128, VW).transpose(1, 0, 2)).astype(F8)
    wo_t = np.ascontiguousarray(
        np.asarray(wo, np.float32).reshape(H // 2, 2, 128, D)
        .transpose(0, 2, 1, 3)).astype(F8)
    wg_t = panelize(np.asarray(w_gate, np.float32) * fnw[:, None], NF).astype(BF16)
    wu_t = panelize(np.asarray(w_up, np.float32) * fnw[:, None], NF).astype(BF16)
    wd_t = panelize(np.asarray(w_down, np.float32), ND).astype(BF16)

    pos_ids = np.asarray(position_ids)

    in_maps = []
    for s in range(N_CORES):
        b, c = divmod(s, NCHUNK)
        lo = c * OWN - (CTX - OWN)          # global start of ctx window
        x_c = np.zeros((CTX, D), np.float32)
        g0, g1 = max(0, lo), c * OWN + OWN
        x_c[g0 - lo: g1 - lo] = x[b, g0:g1]

        posq = np.asarray(pos_ids[b, c * OWN: c * OWN + OWN], np.float64)
        posk_idx = np.clip(np.arange(lo, lo + CTX), 0, S - 1)
        posk = np.asarray(pos_ids[b], np.float64)[posk_idx]
        cosq, sinq = _rope_tables(posq)
        cosk, sink = _rope_tables(posk)

        j = np.arange(CTX)[:, None]         # local key index
        qi = np.arange(OWN)[None, :]
        valid = (j >= qi + 1) & (j <= qi + WINDOW) & (j >= (g0 - lo))
        vt = valid.reshape(NT, 128, OWN)
        mask = np.ascontiguousarray(
            vt[BOUND].astype(F8).transpose(1, 0, 2).reshape(128, len(BOUND) * OWN))
        # per-tile exp bias: interior tiles that are entirely invalid for this
        # core (padding region) get a large negative bias instead of a mask.
        biast = np.full((128, NT), EXP_BIAS, np.float32)
        for t in range(NT):
            if t not in BOUND and not vt[t].any():
                biast[:, t] = -30.0
        # Schraudolph fast-exp bias: int32(x*S + biasf) bitcast f32 ~= e^x
        EXPS = 12102203.161561486
        biasf = (biast * EXPS + (127.0 * 2 ** 23 - 486411.0)).astype(np.float32)

        in_maps.append(dict(
            x_ctx=x_c, wq=wq_t, wk=wk_t, wv=wv_t, wo=wo_t,
            wg=wg_t, wu=wu_t, wd=wd_t,
            cosq=cosq, sinq=sinq, cosk=cosk, sink=sink, mask=mask,
            biast=biast, biasf=biasf))
    return in_maps


_NC_CACHE = {}


def _get_nc(cfg_key='full'):
    if cfg_key not in _NC_CACHE:
        _NC_CACHE[cfg_key] = build_program(FULL)
    return _NC_CACHE[cfg_key]


def kernel(**inputs):
    cfg = FULL
    nc = _get_nc('full')
    in_maps = prep_inputs(cfg, **inputs)
    res = run_bass_kernel_spmd(nc, in_maps, list(range(N_CORES)))
    B, S, D, OWN = cfg['B'], cfg['S'], cfg['D'], cfg['OWN']
    NCHUNK = S // OWN
    out = np.empty((B, S, D), np.float32)
    for s in range(N_CORES):
        b, c = divmod(s, NCHUNK)
        out[b, c * OWN:(c + 1) * OWN] = res.results[s]["y"]
    return out



# revision 54
# speedup vs baseline: 1.0052x; 1.0052x over previous
"""Trainium2 Bass kernel for a hybrid attention+SwiGLU transformer layer.

Strategy: pure data parallelism over B*S = 4096 tokens -> 8 shards of 512.
Each core recomputes K/V over a 1024-token halo (sliding-window attention),
so no collectives are needed. Activations are kept feature-major ("transposed",
[feature, token]) on chip so every projection matmul uses the natural weight
layout as the stationary operand and tokens as the moving free dim (N=512).
Attention scores are computed transposed (scoresT[k, q]) which makes the
whole attention block transpose-free; softmax sums over the partition axis
via ones-matmuls on the PE.

v2 restructure (vs baseline):
 - hT (normed/transposed activations) lives entirely in SBUF; the DRAM
   round-trip and its thousands of tiny descriptors are gone.
 - Phase order P1-own -> Q -> P1-ctx -> KV so Q's matmuls overlap the
   context-halo rmsnorm work.
 - Attention processes 4 heads of one kv-group together, software-pipelined
   by one k-tile so the PE never waits on exp/mask; softmax sums for the 4
   heads pack into one PSUM bank (partition rows 0/32/64/96); interior
   k-tiles skip the mask multiply entirely (per-tile exp bias handles the
   all-or-nothing cases).
 - O-projection loops dc-outer so wo streams exactly once (8MB not 32MB).
 - x2 (post-attention residual) lives in SBUF; n2/fin read it directly.

Matmuls run in bf16 with fp32 PSUM accumulation; norms/softmax/residuals in
fp32. Weights are pre-cast/pre-tiled on host; rotary tables, window masks and
per-tile exp biases are host-precomputed per core.

v3: Q/K/V/O projections, attention AV and the softmax-sum matmuls run in
fp8e4 with DoubleRow perf mode (two contraction rows per PE pass): hT, vP,
attnT and the exp outputs are stored fp8; wq/wk are host-scaled by 64 into
fp8's normal range and descaled for free via the existing rope-path copies.
Scores (q.k) and the whole FFN stay bf16 for accuracy (fp8 FFN busts the
2e-2 tolerance; measured in sim).

v4-v10 pipeline work:
 - sliding-window-aware edges: k-tiles 0/1 (10/11) only score/exp the low
   (high) query half; the dead pm half is memset once.
 - x loads once into the x2 residual buffer (P1-own norms read it, the O
   projection accumulates into it in place); P1-ctx DMA is prefetch-queued.
 - attention groups software-pipeline ACROSS groups: the last two AV/ssum
   pair-drains and the accumulator copy-out are carried into the next
   group's loop (issued behind its independent score matmuls), PSUM state
   is allocated lazily, and softmax normalization is PE-free: one batched
   DVE reciprocal, 1/sum row partition-broadcast by the idle DMA engines
   via a DRAM bounce, final scale on DVE.  Full-width boundary tiles of
   head j=0 use a Schraudolph fast-exp on DVE (fused with the mask mul)
   to offload the scalar engine's exp wall.
 - n2 rmsnorm tiles are emitted as their x2 rows finish; the FFN output
   transposes/stores interleave with the last down-proj matmuls.
"""
import sys, os, math

sys.path.insert(0, '/opt/trn_rl_repo')

import numpy as np
import ml_dtypes

import concourse.bass as bass
import concourse.mybir as mybir
import concourse.tile as tile
from concourse import bacc
from concourse.masks import make_identity
from concourse.bass_utils import run_bass_kernel_spmd

AF = mybir.ActivationFunctionType
DT = mybir.dt
ALU = mybir.AluOpType
BF16 = ml_dtypes.bfloat16

N_CORES = 8
EPS = 1e-6
ROPE_BASE = 10000.0
RD = 64           # rotary dim
WINDOW = 1024
EXP_BIAS = -3.5
QK_SCALE = 64.0   # host-side wq/wk scale into fp8 normal range
F8 = ml_dtypes.float8_e4m3

FULL = dict(D=2048, H=16, KVH=4, FFN=8192, B=2, S=2048, OWN=512, CTX=1536)

# boundary context tiles (those that ever need an elementwise mask)
BOUND = [0, 1, 2, 3, 8, 9, 10, 11]

# quadrant-local 16-row half swap for stream_shuffle (rope pair exchange)
SHUF_MASK = [(i + 16) % 32 for i in range(32)]
# per-head rotary feature permutation: [e0..e15 | o0..o15 | e16..e31 | o16..o31 | 64:]
ROPE_PERM = ([2 * i for i in range(16)] + [2 * i + 1 for i in range(16)]
             + [32 + 2 * i for i in range(16)] + [33 + 2 * i for i in range(16)]
             + list(range(64, 128)))


def build_program(cfg):
    D, H, KVH, FFN = cfg['D'], cfg['H'], cfg['KVH'], cfg['FFN']
    OWN, CTX = cfg['OWN'], cfg['CTX']
    HD = 128
    ND = D // 128            # feature blocks of the model dim
    NF = FFN // 128          # feature blocks of the ffn dim
    NO = OWN // 128          # own token tiles (4)
    NT = CTX // 128          # context token tiles (12)
    NCH = CTX // 512         # context chunks of 512 (3)
    NB = len(BOUND)          # boundary tiles (8)
    FG = 16
    NFG = NF // FG
    VW = KVH * HD
    REP = H // KVH
    f32, bf16, f8 = DT.float32, DT.bfloat16, DT.float8e4
    DR = mybir.MatmulPerfMode.DoubleRow
    rsd = 1.0 / math.sqrt(HD)

    nc = bacc.Bacc("TRN2", target_bir_lowering=False, debug=False,
                   num_devices=N_CORES)

    # ---------------- DRAM I/O ----------------
    x_ctx = nc.dram_tensor("x_ctx", [CTX, D], f32, kind="ExternalInput")
    wq_d = nc.dram_tensor("wq", [H, 128, ND, 128], f8, kind="ExternalInput")
    wk_d = nc.dram_tensor("wk", [KVH, 128, ND, 128], f8, kind="ExternalInput")
    wv_d = nc.dram_tensor("wv", [128, ND, VW], f8, kind="ExternalInput")
    wo_d = nc.dram_tensor("wo", [H // 2, 128, 2, D], f8, kind="ExternalInput")
    wg_d = nc.dram_tensor("wg", [NF, 128, ND * 128], bf16, kind="ExternalInput")
    wu_d = nc.dram_tensor("wu", [NF, 128, ND * 128], bf16, kind="ExternalInput")
    wd_d = nc.dram_tensor("wd", [ND, 128, NF * 128], bf16, kind="ExternalInput")
    cosq_d = nc.dram_tensor("cosq", [64, OWN], bf16, kind="ExternalInput")
    sinq_d = nc.dram_tensor("sinq", [64, OWN], bf16, kind="ExternalInput")
    cosk_d = nc.dram_tensor("cosk", [64, CTX], bf16, kind="ExternalInput")
    sink_d = nc.dram_tensor("sink", [64, CTX], bf16, kind="ExternalInput")
    mask_d = nc.dram_tensor("mask", [128, NB * OWN], f8, kind="ExternalInput")
    bias_d = nc.dram_tensor("biast", [128, NT], f32, kind="ExternalInput")
    biasf_d = nc.dram_tensor("biasf", [128, NT], f32, kind="ExternalInput")
    y_d = nc.dram_tensor("y", [OWN, D], f32, kind="ExternalOutput")
    EXPS = 12102203.161561486        # 2^23 / ln 2 (Schraudolph fast exp)

    from contextlib import ExitStack
    with tile.TileContext(nc) as tc:
        with ExitStack() as ctx:
            pool = lambda *a, **kw: ctx.enter_context(tc.tile_pool(*a, **kw))
            constp = pool(name="const", bufs=1)
            costp = pool(name="cost", bufs=1)     # cos/sin tables
            maskp = pool(name="maskp", bufs=1)
            qTp = pool(name="qT", bufs=1)
            kTp = pool(name="kT", bufs=1)
            vPp = pool(name="vP", bufs=1)
            wpanp = pool(name="wpan", bufs=4)     # streamed weight panels
            smlp = pool(name="sml", bufs=2)
            psp = pool(name="ps", bufs=8, space="PSUM")

            identity_bf = constp.tile([128, 128], bf16, tag="idb")
            make_identity(nc, identity_bf[:])
            identity_f32 = constp.tile([128, 128], f32, tag="idf")
            make_identity(nc, identity_f32[:])
            ones_col = constp.tile([128, 2, 16], f8, tag="ones_col")
            nc.gpsimd.memset(ones_col[:], 1.0)
            ones_row = constp.tile([1, 128], bf16, tag="ones_row")
            nc.gpsimd.memset(ones_row[:], 1.0)
            eps_b = constp.tile([128, 1], f32, tag="eps_b")
            nc.gpsimd.memset(eps_b[:], EPS)

            # rope tables, masks, biases
            cosq = costp.tile([64, OWN], bf16, tag="cq")
            sinq = costp.tile([64, OWN], bf16, tag="sq")
            cosk = costp.tile([64, CTX], bf16, tag="ck")
            sink = costp.tile([64, CTX], bf16, tag="sk")
            nc.sync.dma_start(cosq[:], cosq_d[:])
            nc.sync.dma_start(sinq[:], sinq_d[:])
            nc.sync.dma_start(cosk[:], cosk_d[:])
            nc.sync.dma_start(sink[:], sink_d[:])
            bslot = {t: bi for bi, t in enumerate(BOUND)}

            qT = qTp.tile([128, H * OWN], bf16, tag="qT")
            kT = kTp.tile([128, KVH * CTX], bf16, tag="kT")
            vP = vPp.tile([128, NT, VW], f8, tag="vP")

            # x2 is the post-attention residual buffer; it starts life as the
            # own x rows (loaded once up front, used by P1's own-tile norms),
            # then the O projection accumulates into it in place.
            x2p = pool(name="x2", bufs=1)
            x2 = x2p.tile([128, NO * D], f32, tag="x2")
            for mt in range(NO):
                nc.sync.dma_start(x2[:, mt * D:(mt + 1) * D],
                                  x_ctx[(NT - NO + mt) * 128:
                                        (NT - NO + mt + 1) * 128, :])

            # ====================================================
            # Scope 1: P1-own -> Q -> P1-ctx -> KV   (hT resident)
            # ====================================================
            with tc.tile_pool(name="hT", bufs=1) as hTp, \
                 tc.tile_pool(name="kpan", bufs=4) as kpanp, \
                 tc.tile_pool(name="wvp", bufs=3) as wvp, \
                 tc.tile_pool(name="xf32", bufs=2) as xf32p, \
                 tc.tile_pool(name="hbf", bufs=4) as hbfp, \
                 tc.tile_pool(name="rope", bufs=2) as ropep:

                hT = hTp.tile([128, ND, CTX], f8, tag="hT")

                p1_pre = {}

                def p1_dma(i):
                    """prefetch ctx tile i from DRAM (own tiles live in x2)"""
                    if i >= NT - NO or i in p1_pre:
                        return
                    xt = xf32p.tile([128, D], f32, tag="xf32", bufs=3)
                    nc.sync.dma_start(xt[:], x_ctx[i * 128:(i + 1) * 128, :])
                    p1_pre[i] = xt

                def p1_tile(i):
                    """rmsnorm + transpose ctx tile i into hT.

                    Transposed blocks land packed 8-per-PSUM-bank; the two
                    bank-wide copies out split across scalar and vector so
                    neither engine becomes the per-tile bottleneck.
                    """
                    if i >= NT - NO:
                        xta = x2[:, (i - (NT - NO)) * D:(i - (NT - NO) + 1) * D]
                    else:
                        p1_dma(i)
                        xta = p1_pre.pop(i)[:]
                        p1_dma(i + 1)
                        p1_dma(i + 2)
                    sq = hbfp.tile([128, D], bf16, tag="hbf")
                    ss = smlp.tile([128, 1], f32, tag="ss")
                    nc.scalar.activation(sq[:], xta, AF.Square, accum_out=ss[:])
                    sr = smlp.tile([128, 1], f32, tag="sr")
                    nc.scalar.activation(sr[:], ss[:], AF.Sqrt, scale=1.0 / D,
                                         bias=eps_b[:])
                    rr = smlp.tile([128, 1], f32, tag="rr")
                    nc.vector.reciprocal(rr[:], sr[:])
                    ht = hbfp.tile([128, D], bf16, tag="hbf")
                    nc.vector.tensor_scalar_mul(ht[:], xta, rr[:])
                    # hT free layout within a db block is i*128; 8 consecutive
                    # db blocks of one token tile are NOT contiguous in hT, so
                    # copy out per-db slices from the packed bank.
                    for half in range(2):
                        ptr = psp.tile([128, 1024], bf16, tag="ps")
                        for k in range(8):
                            db = half * 8 + k
                            nc.tensor.transpose(
                                ptr[:, k * 128:(k + 1) * 128],
                                ht[:, db * 128:(db + 1) * 128],
                                identity_bf[:])
                        for k in range(8):
                            db = half * 8 + k
                            dst = hT[:, db, i * 128:(i + 1) * 128]
                            if half == 0:
                                nc.scalar.copy(dst, ptr[:, k * 128:(k + 1) * 128])
                            else:
                                nc.vector.tensor_copy(dst, ptr[:, k * 128:(k + 1) * 128])

                # ---- P1 own tiles (ctx tiles 8..11) ----
                for mt in range(NO):
                    p1_tile(NT - NO + mt)
                for i in range(2):       # prime the ctx-tile prefetch queue
                    p1_dma(i)

                # ---- Q projection + rope, interleaved with P1 ctx tiles so
                # ---- the halo rmsnorm hides under Q's matmul chains ----
                OFF = CTX - OWN
                DSC = 1.0 / QK_SCALE
                for hb in range(H):
                    pan = wpanp.tile([128, ND, 128], f8, tag="wpan")
                    nc.sync.dma_start(pan[:], wq_d[hb])
                    pq = psp.tile([128, OWN], f32, tag="ps")
                    for db in range(0, ND, 2):
                        nc.tensor.matmul(
                            pq[:], pan[:, db:db + 2, :],
                            hT[:, db:db + 2, OFF:OFF + OWN],
                            start=(db == 0), stop=(db == ND - 2),
                            perf_mode=DR)
                    qsl = qT[:, hb * OWN:(hb + 1) * OWN]
                    qstage = ropep.tile([64, OWN], bf16, tag="rst")
                    nc.scalar.mul(qstage[:], pq[0:64, :], DSC)
                    shuf = ropep.tile([64, OWN], bf16, tag="rsh")
                    nc.vector.stream_shuffle(shuf[:], qstage[:], SHUF_MASK)
                    t1 = ropep.tile([64, OWN], bf16, tag="rt1", bufs=1)
                    nc.vector.tensor_mul(t1[:], qstage[:], cosq[:])
                    t2 = ropep.tile([64, OWN], bf16, tag="rt2", bufs=1)
                    nc.vector.tensor_mul(t2[:], shuf[:], sinq[:])
                    nc.vector.tensor_add(qsl[0:64, :], t1[:], t2[:])
                    nc.scalar.mul(qsl[64:128, :], pq[64:128, :], DSC)
                    if hb < NT - NO:
                        p1_tile(hb)

                # ---- K + V over the full context (from hT in SBUF) ----
                kpan = []
                for kb in range(KVH):
                    kp = kpanp.tile([128, ND, 128], f8, tag="kpan")
                    nc.sync.dma_start(kp[:], wk_d[kb])
                    kpan.append(kp)
                for ch in range(NCH):
                    pk = [psp.tile([128, 512], f32, tag="ps", name=f"pk{ch}_{kb}")
                          for kb in range(KVH)]
                    pv = [psp.tile([128, VW], f32, tag="ps", name=f"pv{ch}_{mi}")
                          for mi in range(4)]
                    for db in range(0, ND, 2):
                        hsl = hT[:, db:db + 2, ch * 512:(ch + 1) * 512]
                        wvs = wvp.tile([128, 2, VW], f8, tag="wvs")
                        nc.sync.dma_start(wvs[:], wv_d[:, db:db + 2, :])
                        for kb in range(KVH):
                            nc.tensor.matmul(pk[kb][:],
                                             kpan[kb][:, db:db + 2, :],
                                             hsl,
                                             start=(db == 0), stop=(db == ND - 2),
                                             perf_mode=DR)
                        for mi in range(4):
                            nc.tensor.matmul(pv[mi][:],
                                             hT[:, db:db + 2,
                                                ch * 512 + mi * 128:
                                                ch * 512 + (mi + 1) * 128],
                                             wvs[:],
                                             start=(db == 0), stop=(db == ND - 2),
                                             perf_mode=DR)
                    for mi in range(4):
                        t_idx = ch * 4 + mi
                        nc.vector.tensor_copy(vP[:, t_idx, :], pv[mi][:])
                    for kb in range(KVH):
                        ksl = kT[:, kb * CTX + ch * 512: kb * CTX + (ch + 1) * 512]
                        kstage = ropep.tile([64, 512], bf16, tag="rst")
                        nc.scalar.mul(kstage[:], pk[kb][0:64, :], DSC)
                        shuf = ropep.tile([64, 512], bf16, tag="rsh")
                        nc.vector.stream_shuffle(shuf[:], kstage[:], SHUF_MASK)
                        t1 = ropep.tile([64, 512], bf16, tag="rt1", bufs=1)
                        nc.vector.tensor_mul(t1[:], kstage[:],
                                             cosk[:, ch * 512:(ch + 1) * 512])
                        t2 = ropep.tile([64, 512], bf16, tag="rt2", bufs=1)
                        nc.vector.tensor_mul(t2[:], shuf[:],
                                             sink[:, ch * 512:(ch + 1) * 512])
                        nc.vector.tensor_add(ksl[0:64, :], t1[:], t2[:])
                        nc.scalar.mul(ksl[64:128, :], pk[kb][64:128, :], DSC)

            # ====================================================
            # Attention: 4 heads per kv-group, pipelined by one k-tile
            # ====================================================
            bigB = pool(name="bigB", bufs=1)      # attnT -> gT
            ppp = pool(name="pp", bufs=8)         # small bf16 [128,OWN] tiles
            pbp = pool(name="pbp", bufs=2)        # broadcast 1/ssum tiles
            drp = pool(name="drp", bufs=4, space="DRAM")
            osbp = pool(name="osb", bufs=2)
            stgp = pool(name="stg", bufs=3)       # [128,512] staging
            recpp = pool(name="recp", bufs=2)
            masks = maskp.tile([128, NB * OWN], f8, tag="mask")
            nc.sync.dma_start(masks[:], mask_d[:])
            biast = maskp.tile([128, NT], f32, tag="biast")
            nc.sync.dma_start(biast[:], bias_d[:])
            biasf = maskp.tile([128, NT], f32, tag="biasf")
            nc.sync.dma_start(biasf[:], biasf_d[:])

            attnT = bigB.tile([128, H, OWN], f8, tag="bigB")
            GW = 2                    # heads processed together
            NG = H // GW
            NTP = NT // 2             # context tile pairs (DoubleRow AV)
            # sliding window: query half m01 only sees k-tiles 0..9, half
            # m23 only 2..11 -> edge tiles compute half-width scores/exp and
            # the dead pm half is zeroed once so full-width AV/ssum see 0.
            QR = {0: (0, 256), 1: (0, 256),
                  10: (256, 512), 11: (256, 512)}
            carry = []                # prev group's tail drains (closures)
            deferred = [None, None]   # prev group's finalize closures (a, b)
            for grp in range(NG):
                kb = (grp * GW) // REP
                heads = [grp * GW + j for j in range(GW)]
                # PSUM state allocated lazily (first own drain) so the
                # previous group's banks can free first
                st = {}
                pending = []        # [(tp, [pm pair tiles])] awaiting AV/ssum
                def drain_one(last=False, _st=st, _kb=kb, _pending=pending,
                              _g=grp):
                    if 'ap' not in _st:
                        _st['ap'] = [psp.tile([128, OWN], f32, tag="ps",
                                              name=f"ap{_g % 2}_{j}")
                                     for j in range(GW)]
                        _st['ss'] = [psp.tile([1, OWN], f32, tag="ps",
                                              name=f"ssum{_g % 2}_{j}")
                                     for j in range(GW)]
                    tp_, pms = _pending.pop(0)
                    for j in range(GW):
                        vsl = vP[:, 2 * tp_:2 * tp_ + 2, _kb * HD:(_kb + 1) * HD]
                        nc.tensor.matmul(
                            _st['ap'][j][:], vsl, pms[j][:], start=(tp_ == 0),
                            stop=(last and tp_ == NTP - 1), perf_mode=DR)
                        nc.tensor.matmul(
                            _st['ss'][j][:], ones_col[:, :, 0:1],
                            pms[j][:], start=(tp_ == 0),
                            stop=(last and tp_ == NTP - 1), perf_mode=DR)
                for tp in range(NTP):
                    # prev group's tail drains + PSUM copy-out go first so
                    # its banks free before this pair's score tiles allocate
                    carried = [False]
                    if carry:
                        carry.pop(0)()
                        carried[0] = True
                    pms = [ppp.tile([128, 2, OWN], f8, tag="pt", name=f"pm{j}")
                           for j in range(GW)]
                    # zero the window-dead half of the edge pairs
                    for j in range(GW):
                        if tp == 0:
                            nc.vector.memset(pms[j][:, :, 256:512], 0.0)
                        elif tp == NTP - 1:
                            nc.vector.memset(pms[j][:, :, 0:256], 0.0)
                    for i in range(2):
                        t = 2 * tp + i
                        q0, q1 = QR.get(t, (0, 512))
                        qw = q1 - q0
                        sps = []
                        for j, hb in enumerate(heads):
                            sp = psp.tile([128, OWN], f32, tag="ps")
                            nc.tensor.matmul(
                                sp[:, 0:qw],
                                kT[:, kb * CTX + t * 128: kb * CTX + (t + 1) * 128],
                                qT[:, hb * OWN + q0: hb * OWN + q1],
                                start=True, stop=True)
                            sps.append(sp)
                        for j in range(GW):
                            if t in bslot:
                                if j == 0 and qw == 512:
                                    # Schraudolph fast exp on DVE, fused with
                                    # the mask multiply: int(x*S+B) bitcast
                                    # f32 ~= e^x within ~2%
                                    ti = ppp.tile([128, OWN], DT.int32,
                                                  tag="ti", bufs=2)
                                    nc.vector.tensor_scalar(
                                        ti[:, 0:qw], sps[j][:, 0:qw],
                                        rsd * EXPS, biasf[:, t:t + 1],
                                        ALU.mult, ALU.add)
                                    nc.vector.tensor_mul(
                                        pms[j][:, i, q0:q1],
                                        ti[:, 0:qw].bitcast(f32),
                                        masks[:, bslot[t] * OWN + q0:
                                              bslot[t] * OWN + q1])
                                else:
                                    pt = ppp.tile([128, OWN], bf16, tag="ptb",
                                                  bufs=3)
                                    nc.scalar.activation(
                                        pt[:, 0:qw], sps[j][:, 0:qw],
                                        AF.Exp, scale=rsd,
                                        bias=biast[:, t:t + 1])
                                    nc.vector.tensor_mul(
                                        pms[j][:, i, q0:q1], pt[:, 0:qw],
                                        masks[:, bslot[t] * OWN + q0:
                                              bslot[t] * OWN + q1])
                            else:
                                nc.scalar.activation(pms[j][:, i, :], sps[j][:],
                                                     AF.Exp, scale=rsd,
                                                     bias=biast[:, t:t + 1])
                    # own lag-2 drain and prev group's finalize run after
                    # this pair's (independent) scores
                    if not carried[0]:
                        if tp == 2 and deferred[0] is not None:
                            deferred[0]()
                            deferred[0] = None
                        if tp == 4 and deferred[1] is not None:
                            deferred[1]()
                            deferred[1] = None
                    if len(pending) == 2:
                        drain_one()
                    pending.append((tp, pms))
                # drain down to two pairs; their exps are still in flight, so
                # defer them (and the accumulator/sum copy-out) into the next
                # group's loop where its scores keep the PE busy meanwhile
                while len(pending) > 2:
                    drain_one()
                asbs = [osbp.tile([128, OWN], bf16, tag="osb",
                                  name=f"asb{grp % 2}_{j}")
                        for j in range(GW)]
                ssc = recpp.tile([33, OWN], f32, tag="ssc",
                                 name=f"ssc{grp % 2}")

                def tail1(d=drain_one):
                    d()
                def tail2(d=drain_one, _st=st, _asbs=asbs, _ssc=ssc):
                    d(last=True)
                    # copy PSUM state out right away: frees all 4 banks for
                    # the next group; both sums land quadrant-aligned in one
                    # tile so one per-partition-serial reciprocal serves both
                    for j in range(GW):
                        nc.vector.tensor_copy(_asbs[j][:], _st['ap'][j][:])
                        nc.scalar.copy(_ssc[32 * j:32 * j + 1, :],
                                       _st['ss'][j][:])
                carry = [tail1, tail2]

                # normalization, PE-free: reciprocal on DVE, then the
                # per-query 1/sum row is partition-broadcast by the (idle)
                # DMA engines via a DRAM bounce; final mul on DVE
                pbbs = [pbp.tile([128, OWN], bf16, tag="pbb",
                                 name=f"pbb{grp % 2}_{j}")
                        for j in range(GW)]

                def make_fina(ssc=ssc, pbbs=pbbs, grp=grp):
                    def fina():
                        rec = recpp.tile([33, OWN], bf16, tag="rec", bufs=2)
                        with nc.allow_low_precision(
                                reason="1/ssum broadcast in bf16 is plenty"):
                            nc.vector.reciprocal(rec[:], ssc[:])
                        for j in range(GW):
                            recd = drp.tile([1, OWN], bf16, tag="recd",
                                            name=f"recd{grp % 2}_{j}")
                            nc.sync.dma_start(recd[:],
                                              rec[32 * j:32 * j + 1, :])
                            nc.sync.dma_start(
                                pbbs[j][:], recd[:].to_broadcast((128, OWN)))
                    return fina

                def make_finb(heads=heads, asbs=asbs, pbbs=pbbs):
                    def finb():
                        for j, hb in enumerate(heads):
                            nc.vector.tensor_mul(
                                attnT[:, hb, :], asbs[j][:], pbbs[j][:])
                    return finb
                deferred = [make_fina(), make_finb()]
            for c in carry:
                c()
            deferred[0]()
            deferred[1]()
            deferred = [None, None]

            # ====================================================
            # Scope 2: O projection (+residual) -> x2 (SBUF), n2, FFN, fin
            # ====================================================
            with tc.tile_pool(name="hbf2", bufs=2) as hbfp2:
                gT = bigB.tile([128, ND * OWN], bf16, tag="bigB")

                def n2_tile(mt):
                    """rmsnorm + transpose x2 row-block mt -> gT."""
                    x2t = x2[:, mt * D:(mt + 1) * D]
                    sq = hbfp2.tile([128, D], bf16, tag="hbf")
                    ss = smlp.tile([128, 1], f32, tag="ss")
                    nc.scalar.activation(sq[:], x2t, AF.Square, accum_out=ss[:])
                    sr = smlp.tile([128, 1], f32, tag="sr")
                    nc.scalar.activation(sr[:], ss[:], AF.Sqrt, scale=1.0 / D,
                                         bias=eps_b[:])
                    rr = smlp.tile([128, 1], f32, tag="rr")
                    nc.vector.reciprocal(rr[:], sr[:])
                    gt = hbfp2.tile([128, D], bf16, tag="hbf")
                    nc.vector.tensor_scalar_mul(gt[:], x2t, rr[:])
                    for half in range(2):
                        ptr = psp.tile([128, 1024], bf16, tag="ps")
                        for k in range(8):
                            db = half * 8 + k
                            nc.tensor.transpose(
                                ptr[:, k * 128:(k + 1) * 128],
                                gt[:, db * 128:(db + 1) * 128],
                                identity_bf[:])
                        for k in range(8):
                            db = half * 8 + k
                            dst = gT[:, db * OWN + mt * 128:
                                     db * OWN + (mt + 1) * 128]
                            if half == 0:
                                nc.scalar.copy(dst, ptr[:, k * 128:(k + 1) * 128])
                            else:
                                nc.vector.tensor_copy(dst, ptr[:, k * 128:(k + 1) * 128])

                NDC = D // 512
                for dc in range(NDC):
                    pos = [psp.tile([128, 512], f32, tag="ps",
                                    name=f"po{dc % 2}_{mt}")
                           for mt in range(NO)]
                    for hp in range(H // 2):
                        pan = wpanp.tile([128, 2, 512], f8, tag="wopan")
                        nc.sync.dma_start(
                            pan[:], wo_d[hp][:, :, dc * 512:(dc + 1) * 512])
                        for mt in range(NO):
                            nc.tensor.matmul(
                                pos[mt][:],
                                attnT[:, 2 * hp:2 * hp + 2,
                                      mt * 128:(mt + 1) * 128],
                                pan[:],
                                start=(hp == 0), stop=(hp == H // 2 - 1),
                                perf_mode=DR)
                    for mt in range(NO):
                        xsl = x2[:, mt * D + dc * 512: mt * D + (dc + 1) * 512]
                        nc.vector.tensor_add(xsl, pos[mt][:], xsl)
                        # rmsnorm+transpose of a finished token block overlaps
                        # the remaining adds / FFN weight prefetch
                        if dc == NDC - 1:
                            n2_tile(mt)

                # ---- FFN gate/up/down ----
                with tc.tile_pool(name="acc", bufs=1) as accp, \
                     tc.tile_pool(name="tfg", bufs=1) as tfgp:
                    acc = accp.tile([128, ND * OWN], f32, tag="acc")

                    def fin_og(og):
                        """transpose + final residual -> y for 4 acc blocks;
                        interleaved with the last fg group's down matmuls."""
                        for mt in range(NO):
                            ptg = psp.tile([128, 512], f32, tag="ps")
                            for k in range(4):
                                ob = og * 4 + k
                                nc.tensor.transpose(
                                    ptg[:, k * 128:(k + 1) * 128],
                                    acc[:, ob * OWN + mt * 128:
                                        ob * OWN + (mt + 1) * 128],
                                    identity_f32[:])
                            ys = stgp.tile([128, 512], f32, tag="ys", bufs=2)
                            nc.vector.tensor_add(
                                ys[:], ptg[:],
                                x2[:, mt * D + og * 512: mt * D + (og + 1) * 512])
                            nc.sync.dma_start(
                                y_d[mt * 128:(mt + 1) * 128,
                                    og * 512:(og + 1) * 512], ys[:])

                    for fg in range(NFG):
                        t_fg = tfgp.tile([128, FG * OWN], bf16, tag="tfg")
                        for j in range(FG):
                            fb = fg * FG + j
                            gpan = wpanp.tile([128, ND * 128], bf16, tag="wpan")
                            nc.sync.dma_start(gpan[:], wg_d[fb])
                            upan = wpanp.tile([128, ND * 128], bf16, tag="wpan")
                            nc.sync.dma_start(upan[:], wu_d[fb])
                            pg = psp.tile([128, OWN], f32, tag="ps")
                            pu = psp.tile([128, OWN], f32, tag="ps")
                            for db in range(ND):
                                nc.tensor.matmul(pg[:], gpan[:, db * 128:(db + 1) * 128],
                                                 gT[:, db * OWN:(db + 1) * OWN],
                                                 start=(db == 0), stop=(db == ND - 1))
                                nc.tensor.matmul(pu[:], upan[:, db * 128:(db + 1) * 128],
                                                 gT[:, db * OWN:(db + 1) * OWN],
                                                 start=(db == 0), stop=(db == ND - 1))
                            sg = osbp.tile([128, OWN], bf16, tag="osb")
                            nc.scalar.activation(sg[:], pg[:], AF.Sigmoid)
                            sg2 = ppp.tile([128, OWN], bf16, tag="pt")
                            nc.vector.tensor_mul(sg2[:], sg[:], pg[:])
                            nc.vector.tensor_mul(t_fg[:, j * OWN:(j + 1) * OWN],
                                                 sg2[:], pu[:])
                        for ob in range(ND):
                            dpan = wpanp.tile([128, FG * 128], bf16, tag="wpan")
                            nc.sync.dma_start(
                                dpan[:], wd_d[ob, :, fg * FG * 128:(fg + 1) * FG * 128])
                            pd = psp.tile([128, OWN], f32, tag="ps")
                            for j in range(FG):
                                nc.tensor.matmul(pd[:], dpan[:, j * 128:(j + 1) * 128],
                                                 t_fg[:, j * OWN:(j + 1) * OWN],
                                                 start=(j == 0), stop=(j == FG - 1))
                            osl = acc[:, ob * OWN:(ob + 1) * OWN]
                            if fg == 0:
                                nc.scalar.copy(osl, pd[:])
                            else:
                                nc.vector.tensor_add(osl, osl, pd[:])
                                if fg == NFG - 1 and ob % 4 == 3:
                                    fin_og(ob // 4)

    nc.compile()
    return nc


# ---------------------------------------------------------------------------
# Host-side preparation
# ---------------------------------------------------------------------------

def _rope_tables(pos, dtype=BF16):
    """Build the [64, m] A (cos) and B (+-sin) tables for the permuted layout."""
    inv_freq = 1.0 / (ROPE_BASE ** (np.arange(0, RD, 2, dtype=np.float64) / RD))
    ang = inv_freq[:, None] * pos[None, :].astype(np.float64)   # [32, m]
    cos, sin = np.cos(ang), np.sin(ang)
    rmap = np.concatenate([np.arange(16), np.arange(16),
                           np.arange(16, 32), np.arange(16, 32)])
    sign = np.ones(64); sign[0:16] = -1.0; sign[32:48] = -1.0
    A = cos[rmap]                       # [64, m]
    B = sign[:, None] * sin[rmap]
    return A.astype(dtype), B.astype(dtype)


def prep_inputs(cfg, x, position_ids, attn_norm_w, wq, wk, wv, wo, ffn_norm_w,
                w_gate, w_up, w_down):
    D, H, KVH, FFN = cfg['D'], cfg['H'], cfg['KVH'], cfg['FFN']
    B, S, OWN, CTX = cfg['B'], cfg['S'], cfg['OWN'], cfg['CTX']
    HD = 128
    ND, NF, NT = D // 128, FFN // 128, CTX // 128
    NCHUNK = S // OWN

    x = np.asarray(x, np.float32)
    anw = np.asarray(attn_norm_w, np.float32)
    fnw = np.asarray(ffn_norm_w, np.float32)
    perm = np.asarray(ROPE_PERM)

    def panelize(w, nout):
        # w: [D_in, NOUT*128] -> [NOUT, 128, ND_in*128] panel image
        din = w.shape[0]
        ndin = din // 128
        return np.ascontiguousarray(
            w.reshape(ndin, 128, nout, 128).transpose(2, 1, 0, 3)
            .reshape(nout, 128, ndin * 128))

    wq_f = (np.asarray(wq, np.float32) * anw[:, None]).reshape(D, H, HD)
    wq_f = wq_f[:, :, perm].reshape(D, H * HD) * QK_SCALE
    wq_t = panelize(wq_f, H).astype(F8).reshape(H, 128, ND, 128)
    wk_f = (np.asarray(wk, np.float32) * anw[:, None]).reshape(D, KVH, HD)
    wk_f = wk_f[:, :, perm].reshape(D, KVH * HD) * QK_SCALE
    wk_t = panelize(wk_f, KVH).astype(F8).reshape(KVH, 128, ND, 128)
    VW = KVH * HD
    wv_f = np.asarray(wv, np.float32) * anw[:, None]
    wv_t = np.ascontiguousarray(
        wv_f.reshape(ND, 128, VW).transpose(1, 0, 2)).astype(F8)
    wo_t = np.ascontiguousarray(
        np.asarray(wo, np.float32).reshape(H // 2, 2, 128, D)
        .transpose(0, 2, 1, 3)).astype(F8)
    wg_t = panelize(np.asarray(w_gate, np.float32) * fnw[:, None], NF).astype(BF16)
    wu_t = panelize(np.asarray(w_up, np.float32) * fnw[:, None], NF).astype(BF16)
    wd_t = panelize(np.asarray(w_down, np.float32), ND).astype(BF16)

    pos_ids = np.asarray(position_ids)

    in_maps = []
    for s in range(N_CORES):
        b, c = divmod(s, NCHUNK)
        lo = c * OWN - (CTX - OWN)          # global start of ctx window
        x_c = np.zeros((CTX, D), np.float32)
        g0, g1 = max(0, lo), c * OWN + OWN
        x_c[g0 - lo: g1 - lo] = x[b, g0:g1]

        posq = np.asarray(pos_ids[b, c * OWN: c * OWN + OWN], np.float64)
        posk_idx = np.clip(np.arange(lo, lo + CTX), 0, S - 1)
        posk = np.asarray(pos_ids[b], np.float64)[posk_idx]
        cosq, sinq = _rope_tables(posq)
        cosk, sink = _rope_tables(posk)

        j = np.arange(CTX)[:, None]         # local key index
        qi = np.arange(OWN)[None, :]
        valid = (j >= qi + 1) & (j <= qi + WINDOW) & (j >= (g0 - lo))
        vt = valid.reshape(NT, 128, OWN)
        mask = np.ascontiguousarray(
            vt[BOUND].astype(F8).transpose(1, 0, 2).reshape(128, len(BOUND) * OWN))
        # per-tile exp bias: interior tiles that are entirely invalid for this
        # core (padding region) get a large negative bias instead of a mask.
        biast = np.full((128, NT), EXP_BIAS, np.float32)
        for t in range(NT):
            if t not in BOUND and not vt[t].any():
                biast[:, t] = -30.0
        # Schraudolph fast-exp bias: int32(x*S + biasf) bitcast f32 ~= e^x
        EXPS = 12102203.161561486
        biasf = (biast * EXPS + (127.0 * 2 ** 23 - 486411.0)).astype(np.float32)

        in_maps.append(dict(
            x_ctx=x_c, wq=wq_t, wk=wk_t, wv=wv_t, wo=wo_t,
            wg=wg_t, wu=wu_t, wd=wd_t,
            cosq=cosq, sinq=sinq, cosk=cosk, sink=sink, mask=mask,
            biast=biast, biasf=biasf))
    return in_maps


_NC_CACHE = {}


def _get_nc(cfg_key='full'):
    if cfg_key not in _NC_CACHE:
        _NC_CACHE[cfg_key] = build_program(FULL)
    return _NC_CACHE[cfg_key]


def kernel(**inputs):
    cfg = FULL
    nc = _get_nc('full')
    in_maps = prep_inputs(cfg, **inputs)
    res = run_bass_kernel_spmd(nc, in_maps, list(range(N_CORES)))
    B, S, D, OWN = cfg['B'], cfg['S'], cfg['D'], cfg['OWN']
    NCHUNK = S // OWN
    out = np.empty((B, S, D), np.float32)
    for s in range(N_CORES):
        b, c = divmod(s, NCHUNK)
        out[b, c * OWN:(c + 1) * OWN] = res.results[s]["y"]
    return out

